# revision 21
# baseline (speedup 1.0000x reference)
"""BlockCrossAttention TRN2 Bass kernel — 8-core SPMD with KV AllGather.

Sharding: core c => batch b = c//4, quarter r = c%4.
Host side: compacts encoder tokens by the attention mask (~2056 of 4096
kept; capacity CAP=2304), transposes hs/enc slices, and pre-casts all
matmul operands to bf16.

Per core:
  - pool_avg its 2048 decoder tokens -> pooledT [D, 128 blocks] (bf16)
  - project K^T,V for its OWN quarter of compacted enc tokens (576)
  - 2x AllGather (bf16, ~0.3 MiB in) across the 4 cores of its batch
    -> full K^T [256hd, 2304], V [2304, 256hd]
  - attention for all 16 heads over its own 128 blocks (softmax via a
    mask column appended to V: denominator excludes pad tokens)
  - local out-projection with full Wo -> [128 blocks, 1024]
Host broadcasts block rows back to token level and concatenates.

Numerics: all matmuls bf16 (inputs host-cast), accumulation f32 in
PSUM, exp on ACT in f32->bf16.  Compaction is exact: reference's
masked scores give exp(-1e9)==0 contributions.
"""
import sys

sys.path.insert(0, "/opt/trn_rl_repo")

import numpy as np
import ml_dtypes

import concourse.bass as bass
import concourse.tile as tile
from concourse import bacc, mybir
from concourse.bass import ts
from concourse.bass_utils import run_bass_kernel_spmd
from concourse.masks import make_identity

F32 = mybir.dt.float32
BF16 = mybir.dt.bfloat16

# problem constants (hardcoded per contract)
B, LDEC, LENC, D = 2, 8192, 4096, 1024
BLOCK, H, KV, DH = 16, 16, 4, 64
NB = LDEC // BLOCK            # 512 blocks per batch
NCORES = 8
TOK = LDEC // 4               # 2048 decoder tokens per core
NBQ = NB // 4                 # 128 blocks per core
KD = D // 128                 # 8 chunks of D
CAP = 2304                    # compacted-enc capacity (18 * 128)
CAPQ = CAP // 4               # 576 enc tokens projected per core
EC = CAP // 128               # 18 chunks of 128 enc tokens
HALF = EC // 2                # 9 chunks per exp batch
# pooled is a SUM over 16 tokens (add tree); fold the /16 into the exp scale
SCALE = float(1.0 / (np.sqrt(np.float32(DH)) * BLOCK))

_CACHE = {}
BF = ml_dtypes.bfloat16


def _build():
    nc = bacc.Bacc("TRN2", target_bir_lowering=False, debug=False,
                   num_devices=NCORES)
    hsT = nc.dram_tensor("hsT", [D, TOK], BF16, kind="ExternalInput").ap()
    encTq = nc.dram_tensor("encTq", [D, CAPQ], BF16, kind="ExternalInput").ap()
    maskpm = nc.dram_tensor("maskpm", [128, EC], BF16, kind="ExternalInput").ap()
    wq = nc.dram_tensor("wq", [D, H * DH], BF16, kind="ExternalInput").ap()
    wk = nc.dram_tensor("wk", [D, KV * DH], BF16, kind="ExternalInput").ap()
    wv = nc.dram_tensor("wv", [D, KV * DH], BF16, kind="ExternalInput").ap()
    wo = nc.dram_tensor("wo", [H * DH, D], BF16, kind="ExternalInput").ap()
    outb = nc.dram_tensor("outb", [NBQ, D], F32, kind="ExternalOutput").ap()

    with tile.TileContext(nc) as tc:
        _body(nc, tc, hsT, encTq, maskpm, wq, wk, wv, wo, outb)
    nc.compile()
    return nc


def _body(nc, tc, hsT, encTq, maskpm, wq, wk, wv, wo, outb):
    from contextlib import ExitStack
    groups = [[0, 1, 2, 3], [4, 5, 6, 7]]
    with ExitStack() as ctx:
        pool = lambda name, bufs, **kw: ctx.enter_context(
            tc.tile_pool(name=name, bufs=bufs, **kw))
        constp = pool("const", 1)
        wkvp = pool("wkv", KD)
        encp = pool("enc", KD)
        wqp = pool("wq", KD)
        wop = pool("wo", KD)
        pooledp = pool("pooled", KD)
        qtp = pool("qt", KD)
        ktp = pool("ktsb", 1)
        v5p = pool("v5", EC)
        otp = pool("ot", 1)
        smallp = pool("small", 8)
        dramp = pool("dram", 1, space="DRAM")

        # ---- small consts / kv weights / own-quarter enc ----
        maskbf = constp.tile([128, EC], BF16)
        nc.sync.dma_start(maskbf[:], maskpm[:])
        ident = constp.tile([128, 128], BF16)
        make_identity(nc, ident[:])
        wk_sb, wv_sb, enc_sb = [], [], []
        for k in range(KD):
            t = wkvp.tile([128, KV * DH], BF16, tag="wk", name=f"wk{k}")
            nc.gpsimd.dma_start(t[:], wk[ts(k, 128), :])
            wk_sb.append(t)
        for k in range(KD):
            t = wkvp.tile([128, KV * DH], BF16, tag="wv", name=f"wv{k}")
            nc.gpsimd.dma_start(t[:], wv[ts(k, 128), :])
            wv_sb.append(t)
        for k in range(KD):
            t = encp.tile([128, CAPQ], BF16, tag="enc", name=f"enc{k}")
            nc.sync.dma_start(t[:], encTq[ts(k, 128), :])
            enc_sb.append(t)

        # V5 mask columns can be filled before anything else
        V5 = []
        for c in range(EC):
            t5 = v5p.tile([128, KV * (DH + 1)], BF16, tag="v5", name=f"v5_{c}")
            t5r = t5[:].rearrange("p (g x) -> p g x", x=DH + 1)
            nc.vector.tensor_copy(
                t5r[:, :, DH:DH + 1],
                maskbf[:, c:c + 1].broadcast_to((128, KV, 1)))
            V5.append(t5)

        # ---- K^T and V^T projection on own quarter, staged to DRAM ----
        # kv_in rows: 0:256 = K^T (hd), 256:512 = V^T (hd).  One AllGather.
        kv_in = dramp.tile([4 * 128, CAPQ], BF16, name="kv_in")
        kv_out = nc.dram_tensor("kv_out", [4 * 4 * 128, CAPQ], BF16).ap()

        with tc.tile_pool(name="pk", bufs=2, space="PSUM") as ppk, \
             tc.tile_pool(name="kvst", bufs=4) as kvst:
            for a in range(4):          # 0,1 = K^T halves; 2,3 = V^T halves
                w_sb, mk = (wk_sb, a) if a < 2 else (wv_sb, a - 2)
                ps = ppk.tile([128, CAPQ], F32, tag="pk")
                for k in range(KD):
                    for n0, nw in ((0, 512), (512, CAPQ - 512)):
                        nc.tensor.matmul(ps[:, n0:n0 + nw],
                                         w_sb[k][:, ts(mk, 128)],
                                         enc_sb[k][:, n0:n0 + nw],
                                         start=(k == 0), stop=(k == KD - 1))
                kst = kvst.tile([128, CAPQ], BF16, tag="kst")
                nc.vector.tensor_copy(kst[:], ps[:])
                nc.gpsimd.dma_start(kv_in[ts(a, 128), :], kst[:])

        # ---- single AllGather across the 4 cores of this batch ----
        nc.gpsimd.collective_compute(
            "AllGather", mybir.AluOpType.bypass, replica_groups=groups,
            ins=[kv_in[:].opt()], outs=[kv_out.opt()])

        # ---- pooling + Q^T projection (independent chain) ----
        with tc.tile_pool(name="hst", bufs=3) as hstp, \
             tc.tile_pool(name="padd", bufs=2) as paddp, \
             tc.tile_pool(name="pq", bufs=2, space="PSUM") as ppq:
            pooledT = []
            for k in range(KD):
                ht = hstp.tile([128, TOK], BF16, tag="hst")
                nc.scalar.dma_start(ht[:], hsT[ts(k, 128), :])
                htr = ht[:].rearrange("p (b j) -> p b j", j=BLOCK)
                s1 = paddp.tile([128, NBQ * 8], F32, tag="s1")
                s1r = s1[:].rearrange("p (b j) -> p b j", j=8)
                nc.vector.tensor_add(s1r, htr[:, :, 0:8], htr[:, :, 8:16])
                s2 = paddp.tile([128, NBQ * 4], F32, tag="s2")
                s2r = s2[:].rearrange("p (b j) -> p b j", j=4)
                nc.vector.tensor_add(s2r, s1r[:, :, 0:4], s1r[:, :, 4:8])
                s3 = paddp.tile([128, NBQ * 2], F32, tag="s3")
                s3r = s3[:].rearrange("p (b j) -> p b j", j=2)
                nc.vector.tensor_add(s3r, s2r[:, :, 0:2], s2r[:, :, 2:4])
                pt = pooledp.tile([128, NBQ], BF16, tag="pt", name=f"pt{k}")
                ptr = pt[:].rearrange("p (b j) -> p b j", j=1)
                nc.vector.tensor_add(ptr, s3r[:, :, 0:1], s3r[:, :, 1:2])
                pooledT.append(pt)
            wq_sb = []
            for k in range(KD):
                t = wqp.tile([128, H * DH], BF16, tag="wq", name=f"wq{k}")
                nc.scalar.dma_start(t[:], wq[ts(k, 128), :])
                wq_sb.append(t)
            # qpack[mm][j]: rows 0:64 = q^T of head 8mm+j, rows 64:128 = head
            # 8mm+4+j (pairs heads whose kv groups are 2mm / 2mm+1 so the
            # base partitions line up with the K^T pair tiles).
            qpack = [[qtp.tile([128, NBQ], BF16, tag=f"qp{mm}{j}",
                               name=f"qp{mm}{j}") for j in range(4)]
                     for mm in range(2)]
            for m in range(KD):
                ps = ppq.tile([128, NBQ], F32, tag="pq")
                for k in range(KD):
                    nc.tensor.matmul(ps[:], wq_sb[k][:, ts(m, 128)],
                                     pooledT[k][:],
                                     start=(k == 0), stop=(k == KD - 1))
                for half in range(2):
                    h = 2 * m + half
                    mm, j, dsthalf = h // 8, h % 4, (h % 8) // 4
                    nc.vector.tensor_copy(
                        qpack[mm][j][ts(dsthalf, 64), :], ps[ts(half, 64), :])

        # wo loads (needed only at the end)
        wo_sb = []
        for t in range(KD):
            tl = wop.tile([128, D], BF16, tag="wo", name=f"wo{t}")
            nc.scalar.dma_start(tl[:], wo[ts(t, 128), :])
            wo_sb.append(tl)

        # ---- assemble K^T / V^T pair tiles from the AG output ----
        # tile[mk] rows 0:64 = group 2mk, rows 64:128 = group 2mk+1; cols are
        # the global compacted token index (r*CAPQ + t).  Per-rank slabs are
        # plain 2D [128, CAPQ] reads so each chunk's deps resolve early.
        KT_sb, VT_sb = [], []
        for a in range(4):
            dstl = KT_sb if a < 2 else VT_sb
            kt = ktp.tile([128, CAP], BF16, tag=f"kv{a}", name=f"kv{a}")
            dstl.append(kt)
        for r in range(4):      # band-major so early chunks' deps land first
            for a in range(4):
                kt = (KT_sb + VT_sb)[a]
                nc.sync.dma_start(
                    kt[:, r * CAPQ:(r + 1) * CAPQ],
                    kv_out[r * 512 + a * 128:r * 512 + (a + 1) * 128, :])
        # V5[c] cols (g, 0:64) by transposing V^T chunks on the PE
        with tc.tile_pool(name="ptp", bufs=2, space="PSUM") as pptp:
            for c in range(EC):
                tp = pptp.tile([128, 256], BF16, tag="tp")
                for mv in range(2):
                    nc.tensor.transpose(tp[:, ts(mv, 128)],
                                        VT_sb[mv][:, ts(c, 128)], ident[:])
                t5r = V5[c][:].rearrange("p (g x) -> p g x", x=DH + 1)
                nc.vector.tensor_copy(
                    t5r[:, :, 0:DH],
                    tp[:].rearrange("p (g d) -> p g d", d=DH))

        # ---- attention: per head, exp in 2 batches of 9 chunks ----
        OT = [otp.tile([128, NBQ], BF16, tag=f"ot{t}", name=f"ot{t}")
              for t in range(KD)]
        with tc.tile_pool(name="attn", bufs=4) as attnp, \
             tc.tile_pool(name="psc", bufs=1, space="PSUM") as ppsc, \
             tc.tile_pool(name="pav", bufs=1, space="PSUM") as ppav:
            for mm in range(2):
                for j in range(4):
                    hA, hB = 8 * mm + j, 8 * mm + 4 + j
                    gA, gB = 2 * mm, 2 * mm + 1
                    # avA/avB must live in SEPARATE psum banks: a start=True
                    # matmul clears the whole bank's accumulation state, so
                    # interleaved open accumulation groups cannot share one.
                    avAt = ppav.tile([DH + 1, NBQ], F32, tag="avA")
                    avBt = ppav.tile([DH + 1, NBQ], F32, tag="avB")
                    avA, avB = avAt[:], avBt[:]
                    for half in range(2):
                        scA = ppsc.tile([128, HALF * 128], F32, tag="scA")
                        scB = ppsc.tile([128, HALF * 128], F32, tag="scB")
                        for i in range(HALF):
                            c = half * HALF + i
                            lhs = KT_sb[mm][:, ts(c, 128)]
                            nc.tensor.matmul(scA[:, ts(i, 128)], lhs[0:64, :],
                                             qpack[mm][j][0:64, :],
                                             start=True, stop=True)
                            nc.tensor.matmul(scB[:, ts(i, 128)], lhs[64:128, :],
                                             qpack[mm][j][64:128, :],
                                             start=True, stop=True)
                        eA = attnp.tile([128, HALF * 128], BF16, tag="eA")
                        eB = attnp.tile([128, HALF * 128], BF16, tag="eB")
                        nc.scalar.activation(eA[:], scA[:],
                                             mybir.ActivationFunctionType.Exp,
                                             bias=0.0, scale=SCALE)
                        nc.scalar.activation(eB[:], scB[:],
                                             mybir.ActivationFunctionType.Exp,
                                             bias=0.0, scale=SCALE)
                        for i in range(HALF):
                            c = half * HALF + i
                            nc.tensor.matmul(avA, V5[c][:, ts(gA, DH + 1)],
                                             eA[:, ts(i, 128)],
                                             start=(c == 0), stop=(c == EC - 1))
                            nc.tensor.matmul(avB, V5[c][:, ts(gB, DH + 1)],
                                             eB[:, ts(i, 128)],
                                             start=(c == 0), stop=(c == EC - 1))
                    for h, av in ((hA, avA), (hB, avB)):
                        rec = smallp.tile([1, NBQ], F32, tag="rec")
                        nc.vector.reciprocal(rec[:], av[DH:DH + 1, :])
                        recb = smallp.tile([DH, NBQ], F32, tag="recb")
                        nc.gpsimd.partition_broadcast(recb[:], rec[:])
                        nc.vector.tensor_mul(OT[h // 2][ts(h % 2, 64), :],
                                             av[0:DH, :], recb[:])

        # ---- out projection (local, full Wo) ----
        with tc.tile_pool(name="outsb", bufs=1) as outsbp, \
             tc.tile_pool(name="po", bufs=2, space="PSUM") as ppo:
            osb = outsbp.tile([128, D], F32)
            for n in range(2):
                ps = ppo.tile([128, 512], F32)
                for t in range(KD):
                    nc.tensor.matmul(ps[:], OT[t][:], wo_sb[t][:, ts(n, 512)],
                                     start=(t == 0), stop=(t == KD - 1))
                nc.vector.tensor_copy(osb[:, ts(n, 512)], ps[:])
            nc.sync.dma_start(outb[:], osb[:])


def kernel(hidden_states, encoder_hidden_states, attention_mask, Wq, Wk, Wv, Wo):
    if "nc" not in _CACHE:
        _CACHE["nc"] = _build()
    nc = _CACHE["nc"]

    hs = np.asarray(hidden_states, dtype=np.float32)
    enc = np.asarray(encoder_hidden_states, dtype=np.float32)
    mask = np.asarray(attention_mask)

    # host-side compaction (exact: masked tokens contribute exp(-1e9)==0)
    encT_c, maskpm_b = [], []
    for b in range(B):
        idx = np.nonzero(mask[b])[0]
        assert len(idx) <= CAP, f"mask keeps {len(idx)} > CAP={CAP} tokens"
        ec = np.zeros((CAP, D), dtype=np.float32)
        ec[:len(idx)] = enc[b][idx]
        encT_c.append(np.ascontiguousarray(ec.T.astype(BF)))
        mc = np.zeros((CAP,), dtype=np.float32)
        mc[:len(idx)] = 1.0
        maskpm_b.append(np.ascontiguousarray(mc.reshape(EC, 128).T.astype(BF)))

    wq_b = np.ascontiguousarray(np.asarray(Wq, np.float32).astype(BF))
    wk_b = np.ascontiguousarray(np.asarray(Wk, np.float32).astype(BF))
    wv_b = np.ascontiguousarray(np.asarray(Wv, np.float32).astype(BF))
    wo_b = np.ascontiguousarray(np.asarray(Wo, np.float32).astype(BF))

    in_maps = []
    for c in range(NCORES):
        b, r = c // 4, c % 4
        in_maps.append({
            "hsT": np.ascontiguousarray(
                hs[b, r * TOK:(r + 1) * TOK].T.astype(BF)),
            "encTq": np.ascontiguousarray(
                encT_c[b][:, r * CAPQ:(r + 1) * CAPQ]),
            "maskpm": maskpm_b[b],
            "wq": wq_b,
            "wk": wk_b,
            "wv": wv_b,
            "wo": wo_b,
        })
    res = run_bass_kernel_spmd(nc, in_maps, list(range(NCORES)),
                               **_CACHE.get("run_kwargs", {}))
    _CACHE["last_result"] = res
    blocks = np.empty((B, NB, D), dtype=np.float32)
    for c in range(NCORES):
        b, r = c // 4, c % 4
        blocks[b, r * NBQ:(r + 1) * NBQ] = res.results[c]["outb"]
    out = np.repeat(blocks, BLOCK, axis=1)
    return out
